# revision 2
# baseline (speedup 1.0000x reference)
"""Trainium2 Bass kernel for nn_ASIC_9354438771385 (8 NeuronCores, SPMD).

Math (reference.py): 4 layers; per layer, with a=v[b], c=v[(b+1)%256],
d=v[(b+2)%256] (rolls along batch; KERNEL=(2,2) gives shifts (0,1,1,2)):
    w_p  = f_{b0}(a) f_{b1}(c) f_{b2}(c) f_{b3}(d),   p = 8b0+4b1+2b2+b3,
           f_1(x)=x, f_0(x)=1-x
    out  = clip(sum_p w_p * tw_p[s], 0, 1),  tw = sigmoid(toggle_gates)
and for layers 1..3 a regularizer reg -= mean(log w) + mean(log(1-w)).

Device-side identities used:
  * sum_p w_p = 1 and tw in [0.5, 0.7311) => out is a convex combination of
    tw values, so the clip is a no-op and all post-layer-0 values live in
    [0.5, 0.7311) (log inputs are well-conditioned).
  * mean(log w) = 2*mean(ln(v*(1-v))): each of the 4 kernel slots is a roll
    of v along batch, and a full-batch sum is roll-invariant.
  * Only 12 of the 16 w_p are distinct ((b1,b2)=(0,1),(1,0) coincide), so
    mean(log(1-w)) needs 12 ln's with the q01 group double-weighted.
  * out is evaluated in the monomial basis {1,a} x {1,c,c^2} x {1,d}:
      out = P0 + a*P1,  P_i = Q_i0 + c*Q_i1 + c^2*Q_i2,
      Q_ij = C[i,j,0][s] + C[i,j,1][s]*d
    with the 12 coefficient fields C computed on-chip from tw.

Distribution: H (=64) is sharded over the 8 cores (8 rows each; the rolls
wire neighbors across batch, which every core holds in full, so no
communication is needed). Per core the 512 spatial positions map to
(partition p, group t) = (s % 128, s // 128) and batch lives on the free
axis with a 2-column wraparound halo, so rolls are free-axis offsets.
reg partial sums leave each core as a (128, 2) tile and are combined on the
host during unsharding.
"""

import numpy as np

import concourse.bass as bass
import concourse.bacc as bacc
import concourse.mybir as mybir
from concourse.tile import TileContext
from concourse.bass_utils import run_bass_kernel_spmd

F32 = mybir.dt.float32
F16 = mybir.dt.float16
AF = mybir.ActivationFunctionType
OP = mybir.AluOpType

B, HH, WW = 256, 64, 64
L, NPAT = 4, 16
NCORES = 8
HLOC = HH // NCORES
S = HLOC * WW          # 512 spatial positions per core
T = S // 128           # 4 t-groups
BH = B + 2             # halo'd batch axis


def _bap(t, offset_elems, dims):
    """Raw AP over tile t; dims = [[step, count], ...] (after partitions)."""
    return bass.AP(tensor=t.tensor, offset=t.offset + offset_elems,
                   ap=[list(t.ap[0])] + [list(x) for x in dims])


def _single_act_table():
    """Restrict the act-table chooser to natural_log_exp_and_others so Exp
    (sigmoid) and Ln share one table load."""
    orig = bacc.get_activation_tables
    if getattr(bacc.get_activation_tables, "_asic_patched", False):
        return

    def patched(arch):
        t = orig(arch)
        return {k: (v if k == 'natural_log_exp_and_others' else set())
                for k, v in t.items()}

    patched._asic_patched = True
    bacc.patched_orig_gat = orig
    bacc.get_activation_tables = patched


def build_nc(compute_dtype=F16, split=2):
    CD = compute_dtype
    fp16 = CD != F32
    TC = T // split
    _single_act_table()

    nc = bacc.Bacc("TRN2", target_bir_lowering=False, debug=False,
                   num_devices=NCORES)
    g, v_, s_ = nc.gpsimd, nc.vector, nc.scalar

    x_d = nc.dram_tensor("x", [S, B], F32, kind="ExternalInput")
    tg_d = nc.dram_tensor("tg", [128, L * NPAT * T], F32,
                          kind="ExternalInput")
    out_d = nc.dram_tensor("out", [S, B], F32, kind="ExternalOutput")
    stats_d = nc.dram_tensor("stats", [128, 2], F32, kind="ExternalOutput")

    with TileContext(nc) as tc:
        with (
            tc.tile_pool(name="persist", bufs=1) as persist,
            tc.tile_pool(name="vpool", bufs=3) as vpool,
            tc.tile_pool(name="work", bufs=2) as work,
            tc.tile_pool(name="wpool", bufs=2) as wpool,
            tc.tile_pool(name="scratch", bufs=1) as scratch,
        ):
            # ---- tw = sigmoid(tg) and monomial coefficients C ----
            tg = persist.tile([128, L, NPAT, T], F32)
            nc.sync.dma_start(out=tg, in_=tg_d[:, :].rearrange(
                "p (l q t) -> p l q t", l=L, q=NPAT))
            tw = persist.tile([128, L, NPAT, T], F32)
            s_.activation(tw, tg, AF.Exp, scale=-1.0)
            v_.tensor_scalar_add(tw, tw, 1.0)
            v_.reciprocal(tw, tw)

            Dt = persist.tile([128, L, 8, 2, T], F32)
            twe = tw.rearrange("p l (q e) t -> p l q e t", e=2)
            v_.tensor_copy(Dt[:, :, :, 0, :], twe[:, :, :, 0, :])
            v_.tensor_sub(Dt[:, :, :, 1, :], twe[:, :, :, 1, :],
                          twe[:, :, :, 0, :])
            Gt = persist.tile([128, L, 2, 3, 2, T], F32)
            D_ = Dt.rearrange("p l (b0 bb) k t -> p l b0 bb k t", bb=4)
            tA = scratch.tile([128, L, 2, 2, T], F32)
            v_.tensor_add(tA, D_[:, :, :, 1, :, :], D_[:, :, :, 2, :, :])
            v_.tensor_copy(Gt[:, :, :, 0, :, :], D_[:, :, :, 0, :, :])
            v_.scalar_tensor_tensor(Gt[:, :, :, 1, :, :],
                                    D_[:, :, :, 0, :, :], -2.0, tA,
                                    OP.mult, OP.add)
            z2 = scratch.tile([128, L, 2, 2, T], F32)
            v_.tensor_sub(z2, D_[:, :, :, 3, :, :], tA)
            v_.tensor_add(Gt[:, :, :, 2, :, :], z2, D_[:, :, :, 0, :, :])
            Ct = persist.tile([128, L, 2, 3, 2, T], F32)
            v_.tensor_copy(Ct[:, :, 0], Gt[:, :, 0])
            v_.tensor_sub(Ct[:, :, 1], Gt[:, :, 1], Gt[:, :, 0])

            # ---- v0 ----
            x32 = scratch.tile([128, T, B], F32)
            nc.sync.dma_start(out=x32, in_=x_d[:, :].rearrange(
                "(t p) b -> p t b", p=128))
            R = persist.tile([128, 2, 3, split], F32)
            v_.memset(R, 0.0)

            vas = []
            for h in range(split):
                va = vpool.tile([128, 2, TC, BH], CD, tag=f"v{h}")
                ts = slice(h * TC, (h + 1) * TC)
                v_.tensor_copy(va[:, 0, :, 0:B], x32[:, ts])
                v_.tensor_copy(va[:, 0, :, B:BH], x32[:, ts, 0:2])
                vas.append(va)

            for l in range(L):
                for h in range(split):
                    va = vas[h]
                    v = va[:, 0]
                    a = v[:, :, 0:B]
                    d = v[:, :, 2:B + 2]

                    vc2 = work.tile([128, 2, TC, B], CD, tag=f"vc2_{h}")
                    c, c2 = vc2[:, 0], vc2[:, 1]
                    v_.tensor_copy(c, v[:, :, 1:B + 1])
                    v_.tensor_mul(c2, c, c)

                    Qt = work.tile([128, 2, 3, TC, B], CD, tag=f"Qt_{h}")
                    for i in range(2):
                        for j in range(3):
                            for t in range(TC):
                                tt = h * TC + t
                                v_.tensor_scalar(
                                    Qt[:, i, j, t, :], d[:, t, :],
                                    Ct[:, l, i, j, 1, tt:tt + 1],
                                    Ct[:, l, i, j, 0, tt:tt + 1],
                                    OP.mult, OP.add)
                    mm = work.tile([128, 2, 2, TC, B], CD, tag=f"mm_{h}")
                    cin = _bap(vc2, 0, [[0, 2], [TC * B, 2], [B, TC], [1, B]])
                    v_.tensor_mul(mm, cin, Qt[:, :, 1:3])
                    madd = work.tile([128, 2, TC, B], CD, tag=f"madd_{h}")
                    v_.tensor_add(madd, mm[:, :, 0], mm[:, :, 1])
                    v_.tensor_add(madd, madd, Qt[:, :, 0])
                    P = madd
                    vb = vpool.tile([128, 2, TC, BH], CD, tag=f"v{h}")
                    mo = work.tile([128, TC, B], CD, tag=f"mo_{h}")
                    v_.tensor_mul(mo, a, P[:, 1])
                    v_.tensor_add(vb[:, 0, :, 0:B], P[:, 0], mo)
                    v_.tensor_copy(vb[:, 0, :, B:BH], vb[:, 0, :, 0:2])

                    if l >= 1:
                        li = l - 1
                        vm = va[:, 1]
                        v_.tensor_scalar(vm, v, -1.0, 1.0, OP.mult, OP.add)
                        am = vm[:, :, 0:B]

                        qt3 = work.tile([128, 3, TC, B], CD, tag=f"qt3_{h}")
                        s_.activation(qt3[:, 0], c, AF.Square,
                                      bias=1.0, scale=-1.0)      # (1-c)^2
                        v_.tensor_copy(qt3[:, 1], c2)            # c^2
                        v_.scalar_tensor_tensor(qt3[:, 2], c2, -1.0, c,
                                                OP.mult, OP.add)  # c(1-c)

                        ff = work.tile([128, 2, 2, TC, B], CD, tag=f"ff_{h}")
                        for b0 in range(2):
                            for b3 in range(2):
                                g.tensor_mul(ff[:, b0, b3],
                                             va[:, b0, :, 0:B],
                                             va[:, b3, :, 2:B + 2])

                        wt = wpool.tile([128, 3, 4, TC, B], CD, tag=f"w_{h}")
                        wes = [v_, v_, v_, g]
                        for f in range(4):
                            f_in = _bap(ff, f * TC * B,
                                        [[0, 3], [B, TC], [1, B]])
                            wes[f].tensor_mul(wt[:, :, f], qt3, f_in)

                        lncol = work.tile([128, 4], F32, tag=f"lncol_{h}")
                        w8 = wt[:, 0:2]
                        s_.activation(w8, w8, AF.Ln, bias=1.0, scale=-1.0,
                                      accum_out=lncol[:, 0:1])
                        w4 = wt[:, 2]
                        s_.activation(w4, w4, AF.Ln, bias=1.0, scale=-1.0,
                                      accum_out=lncol[:, 1:2])
                        v_.scalar_tensor_tensor(
                            R[:, 1, li, h:h + 1], lncol[:, 1:2], 2.0,
                            lncol[:, 0:1], OP.mult, OP.add)

                        aam = work.tile([128, TC, B], CD, tag=f"aam_{h}")
                        g.tensor_mul(aam, a, am)
                        s_.activation(aam, aam, AF.Ln,
                                      accum_out=R[:, 0, li, h:h + 1])

                    vas[h] = vb

            # ---- outputs ----
            for h in range(split):
                vv = vas[h][:, 0]
                ts = slice(h * TC, (h + 1) * TC)
                od = out_d[:, :].rearrange("(t p) b -> p t b", p=128)
                if fp16:
                    o32 = scratch.tile([128, TC, B], F32)
                    v_.tensor_copy(o32, vv[:, :, 0:B])
                    nc.sync.dma_start(out=od[:, ts], in_=o32)
                else:
                    nc.sync.dma_start(out=od[:, ts], in_=vv[:, :, 0:B])

            stats = persist.tile([128, 2], F32)
            v_.reduce_sum(stats[:, 0:1], R[:, 0], axis=mybir.AxisListType.XY)
            v_.reduce_sum(stats[:, 1:2], R[:, 1], axis=mybir.AxisListType.XY)
            nc.sync.dma_start(out=stats_d[:, :], in_=stats)

    nc.compile()
    return nc


def shard_inputs(x, toggle_gates):
    in_maps = []
    for k in range(NCORES):
        xs = x[:, k * HLOC:(k + 1) * HLOC, :]
        xs = np.ascontiguousarray(
            xs.transpose(1, 2, 0).reshape(S, B).astype(np.float32))
        tgs = toggle_gates[:, :, k * HLOC:(k + 1) * HLOC, :]
        tgr = tgs.reshape(L, NPAT, T, 2, WW).transpose(3, 4, 0, 1, 2)
        tgr = np.ascontiguousarray(
            tgr.reshape(128, L * NPAT * T).astype(np.float32))
        in_maps.append({"x": xs, "tg": tgr})
    return in_maps


def unshard_outputs(results):
    out_full = np.empty((B, HH, WW), np.float32)
    A = 0.0
    Bs = 0.0
    for k in range(NCORES):
        o = results[k]["out"]
        o = o.reshape(T, 2, WW, B).transpose(3, 0, 1, 2).reshape(B, HLOC, WW)
        out_full[:, k * HLOC:(k + 1) * HLOC, :] = o
        st = results[k]["stats"].astype(np.float64)
        A += st[:, 0].sum()
        Bs += st[:, 1].sum()
    n = float(B * HH * WW)
    reg = -(2.0 * A / n + Bs / (16.0 * n)) / (L - 1)
    return out_full, np.float32(reg)


_NC_CACHE = {}


def get_nc():
    if "nc" not in _NC_CACHE:
        _NC_CACHE["nc"] = build_nc()
    return _NC_CACHE["nc"]


def kernel(x, toggle_gates):
    x = np.asarray(x, dtype=np.float32)
    toggle_gates = np.asarray(toggle_gates, dtype=np.float32)
    nc = get_nc()
    in_maps = shard_inputs(x, toggle_gates)
    res = run_bass_kernel_spmd(nc, in_maps, core_ids=list(range(NCORES)))
    return unshard_outputs(res.results)
